# revision 1
# baseline (speedup 1.0000x reference)
"""Trainium2 Bass kernel for the dense real-space long-range kernel
(N=6144 atoms, B=8 periodic cells, screened-Coulomb pair energy with
minimum-image convention, row-summed per atom).

batch is sorted and cross-graph pairs are masked, so the N x N problem is
block-diagonal over the 8 graphs; one graph per NeuronCore.  Pair math in
fractional coordinates; work is split into (macro row-block, column-chunk)
units, upper-block-triangular (block-row m covers cols >= 126*m):

  w        = wrap(f_j - f_i) - pair formation on DVE/Pool
             ("mod": one tensor_scalar with python_mod; "stt": magic-round
              tensor_scalar + scalar_tensor_tensor)
  group 0:  y = w @ C_bd   (PE)   ; sq = y^2          (ACT Square)
  group1/2: v = w @ G_bd   (PE, G=C C^T); sq = w * v  (DVE stt, one PSUM in)
  q        = ones_bd @ sq  (PE, PSUM accumulate over 3 groups)
  l        = ln(q + soft^2)              (ACT Ln)
  z        = exp(0.5 l + ln sigma)       (ACT Exp)  [= sigma*r]
  n        = z + 0.5 l                   (DVE stt)
  kern     = exp(-n) = exp(-sigma r)/r   (ACT Exp)
  (Ln/Exp/Square share one ACT table -> zero table reloads)
  acc[j] += src_m^T @ kern (PE matvec into pre-zeroed PSUM, start=False)
  ra[i]   = sum_j kern*src_j over cols right of the diagonal block
            (DVE stt with accum_out)
E_i = 0.5*src_i*(acc_i + ra_i) - 0.5*src_i^2*exp(-sigma*soft)/soft  (host)

Units are software-pipelined with LAG=2 (stage2 of unit i emitted with
stage1 of unit i+2); y/q PSUM pools are 3 deep (1 bank each).
"""
import numpy as np

GA = 42            # atoms per k-interleaved row group
ROWS = 3 * GA      # 126 partitions per group tile
GPM = 3            # groups per macro block
MACRO = GA * GPM   # 126 atoms per macro
MAGIC = 12582912.0  # 1.5 * 2**23: (x + MAGIC) - MAGIC == round(x) for |x| < 2**22
NCORES = 8
CHUNK = 512        # PSUM bank / fp32 matmul free-dim limit
MINW = 256         # keep fp32r matmuls at 1 cycle/row
LAG = 3

FORM = "stt"       # "mod": fused wrap via python_mod; "stt": magic round + stt
TRI = True

_cache = {}


def _units(n_macros, cols):
    """(m, ca, cb, mva): compute cols [ca,cb), matvec/reduce from mva.

    Segments are cut at the CHUNK (PSUM bank) grid so matvec outputs never
    cross a bank boundary; short segments merge forward when the result
    still fits a bank-pair span <= CHUNK, and a short head extends left
    (extra compute cols are excluded from the reductions via mva)."""
    units = []
    for m in range(n_macros):
        c0 = MACRO * m if TRI else 0
        pts = ([c0]
               + [p for p in range(((c0 // CHUNK) + 1) * CHUNK, cols, CHUNK)]
               + [cols])
        segs = [[pts[i], pts[i + 1]] for i in range(len(pts) - 1)]
        i = 0
        while i < len(segs) - 1:
            if (segs[i][1] - segs[i][0] < MINW
                    or segs[i + 1][1] - segs[i + 1][0] < MINW) \
                    and segs[i + 1][1] - segs[i][0] <= CHUNK:
                segs[i] = [segs[i][0], segs[i + 1][1]]
                del segs[i + 1]
            else:
                i += 1
        for a, b in segs:
            ca = a if b - a >= MINW else max(0, b - MINW)
            units.append((m, ca, b, a))
    return units


def _mv_pieces(mva, cb_):
    """Split [mva, cb_) at CHUNK-grid (PSUM bank) boundaries."""
    pieces = []
    p = mva
    while p < cb_:
        pn = min(cb_, (p // CHUNK + 1) * CHUNK)
        pieces.append((p, pn))
        p = pn
    return pieces


def _build(n_macros, cols, sigma, soft):
    import concourse.bacc as bacc
    import concourse.mybir as mybir
    import concourse.tile as tile

    f32 = mybir.dt.float32
    f32r = mybir.dt.float32r
    f16 = mybir.dt.float16
    alu = mybir.AluOpType
    act = mybir.ActivationFunctionType

    n_groups = GPM * n_macros
    soft2 = float(np.float32(soft) * np.float32(soft))
    lnsig = float(np.log(np.float64(sigma)))
    krows = ROWS + 1 if FORM == "mod" else ROWS
    units = _units(n_macros, cols)
    nu = len(units)

    nc = bacc.Bacc("TRN2", target_bir_lowering=False, debug=False)

    for name, val in [("soft2", soft2), ("lnsig", lnsig)]:
        t = nc.alloc_sbuf_tensor(f"const-{name}", [128, 1], f32)
        nc.gpsimd.memset(t.ap(), val)
        nc.const_aps.aps[(f32, val)] = t.ap()
    nc.all_engine_barrier()
    # pin the ACT table that serves Ln+Exp+Square so no per-iteration
    # table reloads are needed (act_func_set_id 6 = natural_log_exp_and_others)
    nc.scalar.add_instruction(mybir.InstLoadActFuncSet(
        name=nc.get_next_instruction_name(), act_func_set_id=6, ins=[], outs=[]))

    FB = nc.declare_dram_parameter("FB", [ROWS, cols], f32r, isOutput=False)
    NEGR = nc.declare_dram_parameter("NEGR", [ROWS, n_groups], f32, isOutput=False)
    NEGF = nc.declare_dram_parameter("NEGF", [ROWS, n_groups], f32, isOutput=False)
    CBS = nc.declare_dram_parameter("CBS", [krows, ROWS], f32r, isOutput=False)
    GBS = nc.declare_dram_parameter("GBS", [krows, ROWS], f32r, isOutput=False)
    ONESB = nc.declare_dram_parameter("ONESB", [ROWS, GPM * MACRO], f16, isOutput=False)
    SRCST = nc.declare_dram_parameter("SRCST", [MACRO, n_macros], f16, isOutput=False)
    SRCB = nc.declare_dram_parameter("SRCB", [MACRO, cols], f16, isOutput=False)
    OUTA = nc.declare_dram_parameter("OUTA", [1, cols], f32, isOutput=True)
    OUTR = nc.declare_dram_parameter("OUTR", [MACRO, nu], f32, isOutput=True)

    with tile.TileContext(nc) as tc:
        with tc.tile_pool(name="const", bufs=1) as cpool, \
             tc.tile_pool(name="work", bufs=4) as pool, \
             tc.tile_pool(name="ypsum", bufs=3, space="PSUM") as ypool, \
             tc.tile_pool(name="qpsum", bufs=3, space="PSUM") as qpool, \
             tc.tile_pool(name="apsum", bufs=1, space="PSUM") as apool:
            fb = cpool.tile([ROWS, cols], f32r)
            negr = cpool.tile([ROWS, n_groups], f32)
            negf = cpool.tile([ROWS, n_groups], f32)
            cbs = cpool.tile([krows, ROWS], f32r)
            gbs = cpool.tile([krows, ROWS], f32r)
            onesb = cpool.tile([ROWS, GPM * MACRO], f16)
            srcst = cpool.tile([MACRO, n_macros], f16)
            srcb = cpool.tile([MACRO, cols], f16)
            ra = cpool.tile([MACRO, nu], f32)
            nc.sync.dma_start(negf[:], NEGF[:])
            nc.sync.dma_start(fb[:], FB[:])
            nc.sync.dma_start(cbs[:], CBS[:])
            nc.sync.dma_start(gbs[:], GBS[:])
            nc.sync.dma_start(onesb[:], ONESB[:])
            nc.sync.dma_start(srcst[:], SRCST[:])
            nc.sync.dma_start(srcb[:], SRCB[:])
            nc.gpsimd.memset(ra[:], 0.0)

            if FORM == "mod":
                tmods = [cpool.tile([krows, CHUNK], f32r) for _ in range(GPM)]
                for t in tmods:
                    nc.gpsimd.memset(t[ROWS:krows, :], 1.0)

            acc = apool.tile([1, -(-cols // CHUNK) * CHUNK], f32)
            nc.vector.memset(acc[:], 0.0)

            qmeta = {}

            def stage1(ui):
                m, ca, cb_, mva = units[ui]
                cs = cb_ - ca
                q = qpool.tile([MACRO, CHUNK], f32, tag="q")
                qmeta[ui] = q
                for tgi in range(GPM):
                    g = GPM * m + tgi
                    if FORM == "mod":
                        mov = tmods[tgi]
                        eng = nc.vector if tgi == 0 else nc.gpsimd
                        eng.tensor_scalar(
                            mov[0:ROWS, 0:cs], fb[:, ca:cb_], negr[:, g:g + 1],
                            1.0, alu.add, alu.python_mod)
                        wofs = -0.5
                    else:
                        f = pool.tile([ROWS, CHUNK], f32r, tag=f"f{tgi}")
                        if tgi == 1:
                            nc.scalar.activation(
                                f[:, 0:cs], fb[:, ca:cb_], act.Identity,
                                bias=negf[:, g:g + 1])
                        else:
                            nc.vector.tensor_scalar(
                                f[:, 0:cs], fb[:, ca:cb_], negf[:, g:g + 1],
                                None, alu.add)
                        r = pool.tile([ROWS, CHUNK], f32r, tag=f"r{tgi}")
                        nc.vector.tensor_scalar(
                            r[:, 0:cs], f[:, 0:cs], MAGIC, MAGIC,
                            alu.add, alu.subtract)
                        mov = pool.tile([ROWS, CHUNK], f32r, tag=f"w{tgi}")
                        nc.gpsimd.tensor_tensor(
                            mov[:, 0:cs], f[:, 0:cs], r[:, 0:cs], alu.subtract)
                        wofs = 0.0
                    y = ypool.tile([ROWS, CHUNK], f32, tag="y")
                    use_act_sq = tgi == 0 or (FORM == "stt" and tgi == 2)
                    stat = cbs if use_act_sq else gbs
                    nc.tensor.matmul(y[:, 0:cs], stat[:],
                                     mov[0:krows, 0:cs], start=True, stop=True)
                    sq = pool.tile([ROWS, CHUNK], f16, tag=f"sq{tgi}")
                    if use_act_sq:
                        nc.scalar.activation(sq[:, 0:cs], y[:, 0:cs], act.Square)
                    else:
                        nc.vector.scalar_tensor_tensor(
                            sq[:, 0:cs], mov[0:ROWS, 0:cs], wofs, y[:, 0:cs],
                            alu.add, alu.mult)
                    nc.tensor.matmul(q[:, 0:cs], onesb[:, MACRO * tgi:MACRO * (tgi + 1)],
                                     sq[:, 0:cs],
                                     start=(tgi == 0), stop=(tgi == GPM - 1))

            def stage2(ui):
                m, ca, cb_, mva = units[ui]
                cs = cb_ - ca
                q = qmeta.pop(ui)
                l = pool.tile([MACRO, CHUNK], f16, tag="l")
                nc.scalar.activation(l[:, 0:cs], q[:, 0:cs], act.Ln, bias=soft2)
                z = pool.tile([MACRO, CHUNK], f16, tag="z")
                nc.scalar.activation(z[:, 0:cs], l[:, 0:cs], act.Exp,
                                     bias=lnsig, scale=0.5)
                n = pool.tile([MACRO, CHUNK], f16, tag="n")
                nc.vector.scalar_tensor_tensor(n[:, 0:cs], l[:, 0:cs], 0.5,
                                               z[:, 0:cs], alu.mult, alu.add)
                kern = pool.tile([MACRO, CHUNK], f16, tag="kern")
                nc.scalar.activation(kern[:, 0:cs], n[:, 0:cs], act.Exp,
                                     scale=-1.0)
                pieces = _mv_pieces(mva, cb_)
                for pi, (p0, p1) in enumerate(pieces):
                    nc.tensor.matmul(acc[0:1, p0:p1], srcst[:, m:m + 1],
                                     kern[:, p0 - ca:p1 - ca], start=False,
                                     stop=(ui == nu - 1 and pi == len(pieces) - 1),
                                     skip_group_check=True)
                if TRI:
                    lo = max(MACRO * (m + 1), mva)
                    if lo < cb_:
                        kw = pool.tile([MACRO, CHUNK], f16, tag="kw")
                        nc.vector.scalar_tensor_tensor(
                            kw[:, 0:cb_ - lo], kern[:, lo - ca:cs], 1.0,
                            srcb[:, lo:cb_], alu.mult, alu.mult,
                            accum_out=ra[:, ui:ui + 1])

            for i in range(nu + LAG):
                if i < nu:
                    stage1(i)
                if i >= LAG:
                    stage2(i - LAG)

            eo = pool.tile([1, cols], f32, tag="eo")
            nc.vector.tensor_scalar(eo[:], acc[0:1, 0:cols], 1.0, None, alu.mult)
            nc.sync.dma_start(OUTA[:], eo[:])
            nc.sync.dma_start(OUTR[:], ra[:])
    nc.compile()
    return nc


def _get_program(n_macros, cols, sigma, soft):
    key = (n_macros, cols, round(sigma, 9), round(soft, 9), FORM, TRI)
    if key not in _cache:
        _cache[key] = _build(n_macros, cols, sigma, soft)
    return _cache[key]


LAST_EXEC_TIME_NS = None


def kernel(pos, batch, cell, source, screening, softening, *, _trace=False):
    global LAST_EXEC_TIME_NS
    from concourse.bass_utils import run_bass_kernel_spmd

    pos = np.asarray(pos)
    batch = np.asarray(batch)
    cell = np.asarray(cell)
    source = np.asarray(source, dtype=np.float32)
    sigma = float(np.asarray(screening, dtype=np.float32))
    soft = float(np.asarray(softening, dtype=np.float32))

    n = pos.shape[0]
    nb = cell.shape[0]
    bi = batch.astype(np.int64)
    counts = np.bincount(bi, minlength=nb)
    starts = np.concatenate([[0], np.cumsum(counts)])
    assert nb == NCORES and np.all(np.diff(bi) >= 0)

    # host precompute in float64
    inv = np.linalg.inv(cell.astype(np.float64))
    frac = np.empty((n, 3), dtype=np.float64)
    for g in range(nb):
        i0, i1 = starts[g], starts[g + 1]
        frac[i0:i1] = pos[i0:i1].astype(np.float64) @ inv[g]
    frac32 = frac.astype(np.float32)

    namax = int(counts.max())
    n_macros = -(-namax // MACRO)
    cols = MACRO * n_macros       # padded atom count per core
    n_groups = GPM * n_macros
    diag_c = float(np.exp(-np.float64(sigma) * np.float64(soft)) / np.float64(soft))
    krows = ROWS + 1 if FORM == "mod" else ROWS
    units = _units(n_macros, cols)

    idx_atom = np.arange(ROWS) // 3
    idx_k = np.arange(ROWS) % 3

    in_maps = []
    spads = []
    for g in range(nb):
        i0, i1 = starts[g], starts[g + 1]
        ng = i1 - i0
        fpad = np.zeros((cols, 3), dtype=np.float32)
        fpad[:ng] = frac32[i0:i1]
        spad = np.zeros(cols, dtype=np.float32)
        spad[:ng] = source[i0:i1]
        spads.append(spad)

        fb = np.ascontiguousarray(np.tile(fpad.T, (GA, 1)))  # [126, cols]
        negfa = np.zeros((ROWS, n_groups), dtype=np.float32)
        for t in range(n_groups):
            a = t * GA + idx_atom
            negfa[:, t] = -fpad[a, idx_k]
        C = cell[g].astype(np.float64)
        G = (C @ C.T)
        cbs = np.zeros((krows, ROWS), dtype=np.float32)
        gbs = np.zeros((krows, ROWS), dtype=np.float32)
        for i in range(GA):
            cbs[3 * i:3 * i + 3, 3 * i:3 * i + 3] = C.astype(np.float32)
            gbs[3 * i:3 * i + 3, 3 * i:3 * i + 3] = G.astype(np.float32)
        if FORM == "mod":
            negr = (negfa + np.float32(2.5)).astype(np.float32)
            negf = np.zeros_like(negfa)
            cbs[ROWS, :] = -0.5 * cbs[0:ROWS, :].sum(axis=0)
            gbs[ROWS, :] = -0.5 * gbs[0:ROWS, :].sum(axis=0)
        else:
            negr = (negfa.astype(np.float64) + MAGIC).astype(np.float32)
            negf = negfa
        onesb = np.zeros((ROWS, GPM, MACRO), dtype=np.float32)
        for t in range(GPM):
            for i in range(GA):
                onesb[3 * i:3 * i + 3, t, GA * t + i] = 1.0
        onesb = np.ascontiguousarray(onesb.reshape(ROWS, GPM * MACRO)).astype(np.float16)
        srcst = np.zeros((MACRO, n_macros), dtype=np.float16)
        for m in range(n_macros):
            srcst[:, m] = spad[m * MACRO: m * MACRO + MACRO]
        srcb = np.ascontiguousarray(np.tile(spad[None, :], (MACRO, 1))).astype(np.float16)
        in_maps.append({
            "FB": fb, "NEGR": negr, "NEGF": negf, "CBS": cbs, "GBS": gbs,
            "ONESB": onesb, "SRCST": srcst, "SRCB": srcb,
        })

    nc = _get_program(n_macros, cols, sigma, soft)
    res = run_bass_kernel_spmd(nc, in_maps, list(range(NCORES)), trace=_trace)
    LAST_EXEC_TIME_NS = res.exec_time_ns

    out = np.zeros((n, 1), dtype=np.float32)
    for g in range(nb):
        i0, i1 = starts[g], starts[g + 1]
        ng = i1 - i0
        acc = res.results[g]["OUTA"][0].astype(np.float64)   # [cols]
        tot = acc
        if TRI:
            rag = res.results[g]["OUTR"].astype(np.float64)  # [126, nu]
            rsum = np.zeros(cols, dtype=np.float64)
            for ui, (m, ca, cb_, mva) in enumerate(units):
                lo = max(MACRO * (m + 1), mva)
                if lo < cb_:
                    rsum[m * MACRO:(m + 1) * MACRO] += rag[:, ui]
            tot = acc + rsum
        spad = spads[g].astype(np.float64)
        e = 0.5 * spad * tot - 0.5 * spad * spad * diag_c
        out[i0:i1, 0] = e[:ng].astype(np.float32)
    return out



# revision 3
# speedup vs baseline: 1.2500x; 1.2500x over previous
"""Trainium2 Bass kernel for the dense real-space long-range kernel
(N=6144 atoms, B=8 periodic cells, screened-Coulomb pair energy with
minimum-image convention, row-summed per atom).

batch is sorted and cross-graph pairs are masked, so the N x N problem is
block-diagonal over the 8 graphs; one graph per NeuronCore.  Pair math in
fractional coordinates; work is split into (macro row-block, column-chunk)
units, upper-block-triangular (block-row m covers cols >= 123*m).

Per unit (3 groups of 41 atoms, rows (i,k) k-interleaved):
  groups 0,1 (A-form, aux-row trick):
    f   = fb + negf                      (Pool broadcast-add)
    r   = (f + MAGIC) - MAGIC = round(f) (DVE, exact ints into f32r rw tile)
    y   = WAUX^T @ [r; fb3]              (PE)  = C.fb - C.r   [no bias]
    sq  = Square(y + bias)               (ACT, bias = -(frac_i @ C), f16)
  group 2 (B-form):
    f   = fb + negf                      (DVE ts_ptr)
    r   = magic round                    (DVE)
    w   = f - r                          (Pool tt)
    v   = GBS^T @ w                      (PE, Gram = C C^T)
    sq  = w * v                          (DVE stt, one PSUM operand, f16)
  q   = onesb^T @ sq  (PE, PSUM accumulate over 3 groups)
  l   = ln(q + soft^2)              (ACT Ln)
  z   = exp(0.5 l + ln sigma)       (ACT Exp)  [= sigma*r]
  n   = z + 0.5 l                   (DVE stt)
  kern= exp(-n) = exp(-sigma r)/r   (ACT Exp)
  acc[j] += src_m^T @ kern (PE matvec into pre-zeroed PSUM, start=False)
  ra[i]   = sum_j kern*src_j over cols right of the diagonal block (DVE stt)
E_i = 0.5*src_i*(acc_i + ra_i) - 0.5*src_i^2*exp(-sigma*soft)/soft  (host)

Units are software-pipelined with LAG; sq2/q2 of unit i are emitted with
stage1 of unit i+1 to avoid head-of-line stalls on the w->v->sq2 chain.
"""
import numpy as np

GA = 40            # atoms per k-interleaved row group
ROWS = 3 * GA      # 120 partitions of pair rows per group tile
AUX = 4            # aux rows: 3 fb base rows + 1 zero pad (even fp32r geometry)
GPM = 3            # groups per macro block
MACRO = GA * GPM   # 123 atoms per macro
MAGIC = 12582912.0  # 1.5 * 2**23: (x + MAGIC) - MAGIC == round(x) for |x| < 2**22
NCORES = 8
CHUNK = 512        # PSUM bank / fp32 matmul free-dim limit
MINW = 128
LAG = 2
NPHASE = 3         # rw-tile phases (pipeline depth for full-width moving tiles)

_cache = {}


def _units(n_macros, cols):
    """(m, ca, cb, mva): compute cols [ca,cb), matvec/reduce from mva."""
    units = []
    for m in range(n_macros):
        c0 = MACRO * m
        pts = ([c0]
               + [p for p in range(((c0 // CHUNK) + 1) * CHUNK, cols, CHUNK)]
               + [cols])
        segs = [[pts[i], pts[i + 1]] for i in range(len(pts) - 1)]
        i = 0
        while i < len(segs) - 1:
            if (segs[i][1] - segs[i][0] < MINW
                    or segs[i + 1][1] - segs[i + 1][0] < MINW) \
                    and segs[i + 1][1] - segs[i][0] <= CHUNK:
                segs[i] = [segs[i][0], segs[i + 1][1]]
                del segs[i + 1]
            else:
                i += 1
        for a, b in segs:
            ca = a if b - a >= MINW else max(0, b - MINW)
            units.append((m, ca, b, a))
    return units


def _mv_pieces(mva, cb_):
    """Split [mva, cb_) at CHUNK-grid (PSUM bank) boundaries."""
    pieces = []
    p = mva
    while p < cb_:
        pn = min(cb_, (p // CHUNK + 1) * CHUNK)
        pieces.append((p, pn))
        p = pn
    return pieces


def _build(n_macros, cols, sigma, soft):
    import concourse.bacc as bacc
    import concourse.mybir as mybir
    import concourse.tile as tile

    f32 = mybir.dt.float32
    f32r = mybir.dt.float32r
    f16 = mybir.dt.float16
    alu = mybir.AluOpType
    act = mybir.ActivationFunctionType

    n_groups = GPM * n_macros
    soft2 = float(np.float32(soft) * np.float32(soft))
    lnsig = float(np.log(np.float64(sigma)))
    units = _units(n_macros, cols)
    nu = len(units)
    KR = ROWS + AUX  # 126

    nc = bacc.Bacc("TRN2", target_bir_lowering=False, debug=False)

    for name, val in [("soft2", soft2), ("lnsig", lnsig)]:
        t = nc.alloc_sbuf_tensor(f"const-{name}", [128, 1], f32)
        nc.gpsimd.memset(t.ap(), val)
        nc.const_aps.aps[(f32, val)] = t.ap()
    nc.all_engine_barrier()
    # pin the ACT table serving Ln+Exp+Square (set 6) -> no table reloads
    nc.scalar.add_instruction(mybir.InstLoadActFuncSet(
        name=nc.get_next_instruction_name(), act_func_set_id=6, ins=[], outs=[]))

    FBR = nc.declare_dram_parameter("FBR", [KR, cols], f32r, isOutput=False)
    FBV = nc.declare_dram_parameter("FBV", [ROWS, cols], f32, isOutput=False)
    NEGF = nc.declare_dram_parameter("NEGF", [ROWS, n_groups], f32, isOutput=False)
    WAUX = nc.declare_dram_parameter("WAUX", [KR, ROWS], f32r, isOutput=False)
    GBS = nc.declare_dram_parameter("GBS", [ROWS, ROWS], f32r, isOutput=False)
    BIASP = nc.declare_dram_parameter("BIASP", [ROWS, n_groups], f32, isOutput=False)
    ONESB = nc.declare_dram_parameter("ONESB", [ROWS, GPM * MACRO], f16, isOutput=False)
    SRCST = nc.declare_dram_parameter("SRCST", [MACRO, n_macros], f16, isOutput=False)
    SRCB = nc.declare_dram_parameter("SRCB", [MACRO, cols], f16, isOutput=False)
    OUTA = nc.declare_dram_parameter("OUTA", [1, cols], f32, isOutput=True)
    OUTR = nc.declare_dram_parameter("OUTR", [MACRO, nu], f32, isOutput=True)

    with tile.TileContext(nc) as tc:
        with tc.tile_pool(name="const", bufs=1) as cpool, \
             tc.tile_pool(name="work", bufs=3) as pool, \
             tc.tile_pool(name="ypsum", bufs=4, space="PSUM") as ypool, \
             tc.tile_pool(name="qpsum", bufs=2, space="PSUM") as qpool, \
             tc.tile_pool(name="apsum", bufs=1, space="PSUM") as apool:
            fbr = cpool.tile([KR, cols], f32r)
            fbv = cpool.tile([ROWS, cols], f32)
            negf = cpool.tile([ROWS, n_groups], f32)
            waux = cpool.tile([KR, ROWS], f32r)
            gbs = cpool.tile([ROWS, ROWS], f32r)
            biasp = cpool.tile([ROWS, n_groups], f32)
            onesb = cpool.tile([ROWS, GPM * MACRO], f16)
            srcst = cpool.tile([MACRO, n_macros], f16)
            srcb = cpool.tile([MACRO, cols], f16)
            ra = cpool.tile([MACRO, nu], f32)
            nc.sync.dma_start(fbr[:], FBR[:])
            nc.sync.dma_start(fbv[:], FBV[:])
            nc.sync.dma_start(negf[:], NEGF[:])
            nc.sync.dma_start(waux[:], WAUX[:])
            nc.sync.dma_start(gbs[:], GBS[:])
            nc.sync.dma_start(biasp[:], BIASP[:])
            nc.sync.dma_start(onesb[:], ONESB[:])
            nc.sync.dma_start(srcst[:], SRCST[:])
            nc.sync.dma_start(srcb[:], SRCB[:])
            nc.gpsimd.memset(ra[:], 0.0)

            # full-width A-form moving tiles; aux rows = fb base rows (+zero
            # pad row), DMAed once
            rw = [[cpool.tile([KR, cols], f32r, name=f"rw{g}_{p}")
                   for p in range(NPHASE)] for g in range(2)]
            for g in range(2):
                for p in range(NPHASE):
                    nc.sync.dma_start(rw[g][p][ROWS:KR, :], FBR[ROWS:KR, :])

            acc = apool.tile([1, -(-cols // CHUNK) * CHUNK], f32)
            nc.vector.memset(acc[:], 0.0)

            qmeta = {}
            s2meta = {}

            def stage1(ui):
                m, ca, cb_, mva = units[ui]
                cs = cb_ - ca
                ph = ui % NPHASE
                q = qpool.tile([MACRO, CHUNK], f32, tag="q")
                # B-form f (DVE, no deps) first so DVE starts immediately
                g2 = GPM * m + 2
                f2 = pool.tile([ROWS, CHUNK], f32, tag="f2")
                nc.vector.tensor_scalar(
                    f2[:, 0:cs], fbv[:, ca:cb_], negf[:, g2:g2 + 1], None, alu.add)
                # A-form groups 0,1
                ys = []
                for tgi in range(2):
                    g = GPM * m + tgi
                    fa = pool.tile([ROWS, CHUNK], f32, tag=f"fa{tgi}")
                    nc.gpsimd.tensor_tensor(
                        fa[:, 0:cs], fbv[:, ca:cb_],
                        negf[:, g:g + 1].to_broadcast([ROWS, cs]), alu.add)
                    rt = rw[tgi][ph]
                    nc.vector.tensor_scalar(
                        rt[0:ROWS, ca:cb_], fa[:, 0:cs], MAGIC, MAGIC,
                        alu.add, alu.subtract)
                    y = ypool.tile([MACRO, CHUNK], f32, tag="y")
                    nc.tensor.matmul(y[:, 0:cs], waux[:], rt[:, ca:cb_],
                                     start=True, stop=True)
                    ys.append((g, y))
                # B-form round + w + v
                r2 = pool.tile([ROWS, CHUNK], f32, tag="r2")
                nc.vector.tensor_scalar(
                    r2[:, 0:cs], f2[:, 0:cs], MAGIC, MAGIC, alu.add, alu.subtract)
                w2 = pool.tile([ROWS, CHUNK], f32r, tag="w2")
                nc.gpsimd.tensor_tensor(
                    w2[:, 0:cs], f2[:, 0:cs], r2[:, 0:cs], alu.subtract)
                v2 = ypool.tile([MACRO, CHUNK], f32, tag="y")
                nc.tensor.matmul(v2[:, 0:cs], gbs[:], w2[0:ROWS, 0:cs],
                                 start=True, stop=True)
                # A-form squares + q accumulation for groups 0,1
                for tgi, (g, y) in enumerate(ys):
                    sq = pool.tile([ROWS, CHUNK], f16, tag=f"sq{tgi}")
                    nc.scalar.activation(sq[:, 0:cs], y[:, 0:cs], act.Square,
                                         bias=biasp[:, g:g + 1])
                    nc.tensor.matmul(q[:, 0:cs],
                                     onesb[:, MACRO * tgi:MACRO * (tgi + 1)],
                                     sq[:, 0:cs], start=(tgi == 0), stop=False)
                qmeta[ui] = q
                s2meta[ui] = (w2, v2)

            def stage1b(ui):
                # sq2/q2 of unit ui, emitted one unit later (w->v chain slack)
                m, ca, cb_, mva = units[ui]
                cs = cb_ - ca
                q = qmeta[ui]
                w2, v2 = s2meta.pop(ui)
                sq2 = pool.tile([ROWS, CHUNK], f16, tag="sq2")
                nc.vector.scalar_tensor_tensor(
                    sq2[:, 0:cs], w2[0:ROWS, 0:cs], 0.0, v2[:, 0:cs],
                    alu.add, alu.mult)
                nc.tensor.matmul(q[:, 0:cs], onesb[:, 2 * MACRO:3 * MACRO],
                                 sq2[:, 0:cs], start=False, stop=True)

            def stage2(ui):
                m, ca, cb_, mva = units[ui]
                cs = cb_ - ca
                q = qmeta.pop(ui)
                l = pool.tile([MACRO, CHUNK], f16, tag="l")
                nc.scalar.activation(l[:, 0:cs], q[:, 0:cs], act.Ln, bias=soft2)
                z = pool.tile([MACRO, CHUNK], f16, tag="z")
                nc.scalar.activation(z[:, 0:cs], l[:, 0:cs], act.Exp,
                                     bias=lnsig, scale=0.5)
                n = pool.tile([MACRO, CHUNK], f16, tag="n")
                nc.vector.scalar_tensor_tensor(n[:, 0:cs], l[:, 0:cs], 0.5,
                                               z[:, 0:cs], alu.mult, alu.add)
                kern = pool.tile([MACRO, CHUNK], f16, tag="kern")
                nc.scalar.activation(kern[:, 0:cs], n[:, 0:cs], act.Exp,
                                     scale=-1.0)
                pieces = _mv_pieces(mva, cb_)
                for pi, (p0, p1) in enumerate(pieces):
                    nc.tensor.matmul(acc[0:1, p0:p1], srcst[:, m:m + 1],
                                     kern[:, p0 - ca:p1 - ca], start=False,
                                     stop=(ui == nu - 1 and pi == len(pieces) - 1),
                                     skip_group_check=True)
                lo = max(MACRO * (m + 1), mva)
                if lo < cb_:
                    kw = pool.tile([MACRO, CHUNK], f16, tag="kw")
                    nc.vector.scalar_tensor_tensor(
                        kw[:, 0:cb_ - lo], kern[:, lo - ca:cs], 1.0,
                        srcb[:, lo:cb_], alu.mult, alu.mult,
                        accum_out=ra[:, ui:ui + 1])

            for i in range(nu + 1 + LAG):
                if i < nu:
                    stage1(i)
                if 1 <= i <= nu:
                    stage1b(i - 1)
                if i >= 1 + LAG:
                    stage2(i - 1 - LAG)

            eo = pool.tile([1, cols], f32, tag="eo")
            nc.vector.tensor_scalar(eo[:], acc[0:1, 0:cols], 1.0, None, alu.mult)
            nc.sync.dma_start(OUTA[:], eo[:])
            nc.sync.dma_start(OUTR[:], ra[:])
    nc.compile()
    return nc


def _get_program(n_macros, cols, sigma, soft):
    key = (n_macros, cols, round(sigma, 9), round(soft, 9))
    if key not in _cache:
        _cache[key] = _build(n_macros, cols, sigma, soft)
    return _cache[key]


LAST_EXEC_TIME_NS = None


def kernel(pos, batch, cell, source, screening, softening, *, _trace=False):
    global LAST_EXEC_TIME_NS
    from concourse.bass_utils import run_bass_kernel_spmd

    pos = np.asarray(pos)
    batch = np.asarray(batch)
    cell = np.asarray(cell)
    source = np.asarray(source, dtype=np.float32)
    sigma = float(np.asarray(screening, dtype=np.float32))
    soft = float(np.asarray(softening, dtype=np.float32))

    n = pos.shape[0]
    nb = cell.shape[0]
    bi = batch.astype(np.int64)
    counts = np.bincount(bi, minlength=nb)
    starts = np.concatenate([[0], np.cumsum(counts)])
    assert nb == NCORES and np.all(np.diff(bi) >= 0)

    # host precompute in float64
    inv = np.linalg.inv(cell.astype(np.float64))
    frac = np.empty((n, 3), dtype=np.float64)
    for g in range(nb):
        i0, i1 = starts[g], starts[g + 1]
        frac[i0:i1] = pos[i0:i1].astype(np.float64) @ inv[g]

    namax = int(counts.max())
    n_macros = -(-namax // MACRO)
    cols = namax + (namax % 2)    # columns trimmed to real max atoms (even)
    rows_tot = MACRO * n_macros   # row padding to full macros
    n_groups = GPM * n_macros
    diag_c = float(np.exp(-np.float64(sigma) * np.float64(soft)) / np.float64(soft))
    units = _units(n_macros, cols)
    nu = len(units)
    KR = ROWS + AUX

    idx_atom = np.arange(ROWS) // 3
    idx_k = np.arange(ROWS) % 3

    in_maps = []
    spads = []
    for g in range(nb):
        i0, i1 = starts[g], starts[g + 1]
        ng = i1 - i0
        fpad = np.zeros((rows_tot, 3), dtype=np.float64)
        fpad[:ng] = frac[i0:i1]
        fpad32 = fpad.astype(np.float32)
        spad = np.zeros(rows_tot, dtype=np.float32)
        spad[:ng] = source[i0:i1]
        spads.append(spad)

        fbr = np.zeros((KR, cols), dtype=np.float32)
        fbr[:ROWS] = np.tile(fpad32[:cols].T, (GA, 1))
        fbr[ROWS:ROWS + 3] = fpad32[:cols].T
        fbv = np.ascontiguousarray(np.tile(fpad32[:cols].T, (GA, 1)))      # [123, cols]
        negfa = np.zeros((ROWS, n_groups), dtype=np.float32)
        biasp = np.zeros((ROWS, n_groups), dtype=np.float32)
        C = cell[g].astype(np.float64)
        G = (C @ C.T)
        pseudo = fpad @ C            # ~pos of each padded atom, f64
        for t in range(n_groups):
            a = t * GA + idx_atom
            negfa[:, t] = -fpad32[a, idx_k]
            biasp[:, t] = (-pseudo[a, idx_k]).astype(np.float32)
        waux = np.zeros((KR, ROWS), dtype=np.float32)
        gbs = np.zeros((ROWS, ROWS), dtype=np.float32)
        C32 = C.astype(np.float32)
        G32 = G.astype(np.float32)
        for i in range(GA):
            waux[3 * i:3 * i + 3, 3 * i:3 * i + 3] = -C32
            gbs[3 * i:3 * i + 3, 3 * i:3 * i + 3] = G32
            waux[ROWS:ROWS + 3, 3 * i:3 * i + 3] = C32
        onesb = np.zeros((ROWS, GPM, MACRO), dtype=np.float32)
        for t in range(GPM):
            for i in range(GA):
                onesb[3 * i:3 * i + 3, t, GA * t + i] = 1.0
        onesb = np.ascontiguousarray(onesb.reshape(ROWS, GPM * MACRO)).astype(np.float16)
        srcst = np.zeros((MACRO, n_macros), dtype=np.float16)
        for m in range(n_macros):
            srcst[:, m] = spad[m * MACRO: m * MACRO + MACRO]
        srcb = np.ascontiguousarray(np.tile(spad[None, :cols], (MACRO, 1))).astype(np.float16)
        in_maps.append({
            "FBR": fbr, "FBV": fbv, "NEGF": negfa, "WAUX": waux, "GBS": gbs,
            "BIASP": biasp, "ONESB": onesb, "SRCST": srcst, "SRCB": srcb,
        })

    nc = _get_program(n_macros, cols, sigma, soft)
    res = run_bass_kernel_spmd(nc, in_maps, list(range(NCORES)), trace=_trace)
    LAST_EXEC_TIME_NS = res.exec_time_ns

    out = np.zeros((n, 1), dtype=np.float32)
    for g in range(nb):
        i0, i1 = starts[g], starts[g + 1]
        ng = i1 - i0
        acc = res.results[g]["OUTA"][0].astype(np.float64)   # [cols]
        rag = res.results[g]["OUTR"].astype(np.float64)      # [123, nu]
        rsum = np.zeros(MACRO * n_macros, dtype=np.float64)
        for ui, (m, ca, cb_, mva) in enumerate(units):
            lo = max(MACRO * (m + 1), mva)
            if lo < cb_:
                rsum[m * MACRO:(m + 1) * MACRO] += rag[:, ui]
        tot = np.concatenate([acc, np.zeros(MACRO * n_macros - cols)]) + rsum
        spad = spads[g].astype(np.float64)
        e = 0.5 * spad * tot - 0.5 * spad * spad * diag_c
        out[i0:i1, 0] = e[:ng].astype(np.float32)
    return out
